# revision 15
# baseline (speedup 1.0000x reference)
"""Single-head cross-attention block on 8 NeuronCores (Trainium2, Bass/Tile).

Problem:  out = x + softmax((x@Wq.T+bq) @ (x@Wk.T+bk).T / sqrt(D)) @ (x@Wv.T+bv)
          x: [8, 4096, 256] f32.  Data-parallel: one batch element per core.

Design (see kernel_base.py for the 219-221us baseline rationale; this
revision keeps its numerics bit-identical and attacks PE idle):
  1. Scores algebra: z = (x@A + w) @ x.T, A = 16*scale*Wq.T@Wk host-folded.
  2. P@V in fp8e4 DoubleRow; P = exp(z/16 - C*ln2) straight to fp8; row-sum
     rides as a ones-column in v.  v fp8 in a Hadamard-rotated basis.
  3. exp split 3:1 ScalarE (exact) : VectorE (uint8 Schraudolph exp2).
  4. Trace-driven changes vs baseline (each kills a measured PE stall):
     - head DMA doubled to xT cols 0:1024 both planes (+Apk): blk0's PE
       work no longer outruns the xT chunk DMAs at sk=4..7 (was a 3.7us
       LDWEIGHTS stall at t~7-11us); remaining chunks reordered 1024:2560
       first, 0:1024 refill last.
     - y_proj(blk+1) issued at sk==10 (was 16): its PSUM slots sit in the
       score rotation ring, and at sk==16 the DVE evac queue is long, so
       pair MMs stalled ~1.6us/block waiting the y evac.  At sk==10 the
       DVE queue is empty (pair 3's exp just drained).
     - output DMA triggers moved off the ScalarE queue (sync+gpsimd now):
       a 619ns trigger between exp ACTIVATEs delayed evacs -> rotation
       stalls.
     - exp act-table warmup at t~0 (ebias memset on the vector queue, which
       is free early; gpsimd has ~4us of framework preamble).
     - vp ones-columns: one 2-plane strided memset per pair (was 2).
"""

import numpy as np
import ml_dtypes
from contextlib import ExitStack

import concourse.bass as bass
import concourse.mybir as mybir
import concourse.tile as tile
from concourse import bacc
from concourse.bass_utils import run_bass_kernel_spmd

B, S, D = 8, 4096, 256
P = 128
SQB = 512               # sq block width
NBLK = S // SQB         # 8
NSK = S // P            # 32 sk tiles
NPAIR = NSK // 2        # 16 fp8 DoubleRow pairs
NSUB = SQB // P         # 4
VW = D + 1              # v columns + ones column
SKEW = 3                # PV pair pipeline skew (in pairs)
HEADC = 1024            # head-DMA xT columns per plane
C_SHIFT = 7.4           # P = 2^(z' - C)
EXP_SCALE = 1.0 / 16.0
EXP_BIAS = float(-C_SHIFT * np.log(2.0))
L2E = 1.4426950408889634
TRICK_A = 8.0 * L2E / 16.0                  # uint8 exp2 bit-trick slope
TRICK_B = 56.0 - 8.0 * C_SHIFT - 0.344      # offset, mean-error-centered

F32 = mybir.dt.float32
BF16 = mybir.dt.bfloat16
FP8 = mybir.dt.float8e4
DRM = mybir.MatmulPerfMode.DoubleRow
AF = mybir.ActivationFunctionType

DVE_EXP = True
_NC_CACHE = None


def _col_ap(vec_ap):
    return bass.AP(tensor=vec_ap.tensor, offset=vec_ap.offset,
                   ap=[vec_ap.ap[0], [0, 1]])


def _pl_ap(t, off, n, plane_stride):
    """[128, 2, n] AP over tile t: planes at element offsets off, off+plane_stride."""
    base = t[:, 0:1]
    return bass.AP(tensor=base.tensor, offset=base.offset + off,
                   ap=[base.ap[0], [plane_stride, 2], [1, n]])


def _build():
    global _NC_CACHE
    if _NC_CACHE is not None:
        return _NC_CACHE

    nc = bacc.Bacc("TRN2")
    xb = nc.dram_tensor("xb", [S, D], F32, kind="ExternalInput")      # x + bv
    xTh = nc.dram_tensor("xT", [P, 2 * S], BF16, kind="ExternalInput")  # packed x.T
    Ah = nc.dram_tensor("Apk", [P, 2 * D], BF16, kind="ExternalInput")
    Wh = nc.dram_tensor("Wpk", [P, 2 * D], BF16, kind="ExternalInput")  # Wv.T @ R
    Rh = nc.dram_tensor("Rtpk", [P, 2 * D], BF16, kind="ExternalInput")  # R.T
    wh = nc.dram_tensor("wcol", [D], F32, kind="ExternalInput")
    eyeh = nc.dram_tensor("eye", [P, P], BF16, kind="ExternalInput")
    hh = nc.dram_tensor("head", [P, 3 * 512], BF16, kind="ExternalInput")
    h2h = nc.dram_tensor("head2", [P, 2 * 512], BF16, kind="ExternalInput")
    out = nc.dram_tensor("out", [S, D], F32, kind="ExternalOutput")

    with tile.TileContext(nc) as tc, ExitStack() as ctx:
        persist = ctx.enter_context(tc.tile_pool(name="persist", bufs=1))
        psum = ctx.enter_context(tc.tile_pool(name="psum", bufs=1, space="PSUM"))
        ptp = ctx.enter_context(tc.tile_pool(name="ptp", bufs=8))
        epi = ctx.enter_context(tc.tile_pool(name="epi", bufs=8))

        # --- warmup: pull the exp act-table load (~1.3us) into the DMA
        #     shadow at t=0 ---
        ebias = persist.tile([P, 1], F32, tag="ebias", name="ebias")
        nc.vector.memset(ebias, EXP_BIAS)
        wro = persist.tile([P, 1], F32, tag="wro", name="wro")
        nc.scalar.activation(out=wro, in_=ebias, func=AF.Exp,
                             scale=EXP_SCALE, bias=ebias[:, :])

        # --- startup-critical DMAs.  Engine preambles delay the first
        #     trigger to ~6.2us and desc-gen costs ~0.65us per trigger, so
        #     the FIRST transfer must be small: head1 = [xT 0:512 both
        #     planes | Apk] feeds y_proj + sk<4; head2 = [xT 512:1024]
        #     follows; then the remaining chunks, 0:1024 refill last ---
        xTp = persist.tile([P, 2 * S], BF16, tag="xTp", name="xTp")
        bounds = [0, HEADC, 2560, S]

        def xchunk(pl, ch):
            lo, hi = bounds[ch], bounds[ch + 1]
            eng = nc.scalar if pl else nc.sync
            eng.dma_start(out=xTp[:, pl * S + lo: pl * S + hi],
                          in_=xTh[:, pl * S + lo: pl * S + hi])

        head = persist.tile([P, 3 * 512], BF16, tag="head", name="head")
        nc.sync.dma_start(out=head, in_=hh[:, :])
        wc = []
        for et in range(2):
            t = persist.tile([P, 1], F32, tag=f"wc{et}", name=f"wc{et}")
            nc.sync.dma_start(out=t, in_=_col_ap(wh[et * P:(et + 1) * P]))
            wc.append(t)
        head2 = persist.tile([P, 2 * 512], BF16, tag="head2", name="head2")
        nc.sync.dma_start(out=head2, in_=h2h[:, :])
        Wpk = persist.tile([P, 2 * D], BF16, tag="Wpk", name="Wpk")
        nc.scalar.dma_start(out=Wpk, in_=Wh[:, :])
        Apk = head[:, 2 * 512: 3 * 512]
        xchunk(1, 1)
        xchunk(0, 1)
        xchunk(1, 2)
        xchunk(0, 2)
        Rtpk = persist.tile([P, 2 * D], BF16, tag="Rtpk", name="Rtpk")
        nc.scalar.dma_start(out=Rtpk, in_=Rh[:, :])
        xchunk(0, 0)
        xchunk(1, 0)
        eye = persist.tile([P, P], BF16, tag="eye", name="eye")
        nc.gpsimd.dma_start(out=eye, in_=eyeh[:, :])

        # --- residual (x + bv): one big tile, one SWDGE transfer (issued at
        #     blk0 sk=0 so it doesn't clog startup descriptor generation) ---
        xbig = persist.tile([P, NSK * D], F32, tag="xbig", name="xbig")
        xb_col = xb[0:P, 0:1]
        xb_src = bass.AP(tensor=xb_col.tensor, offset=xb_col.offset,
                         ap=[xb_col.ap[0], [P * D, NSK], [1, D]])
        xbt = [xbig[:, st * D:(st + 1) * D] for st in range(NSK)]

        yTp = persist.tile([P, 2 * S], BF16, tag="yTp", name="yTp")
        vp = [persist.tile([P, 2 * VW], FP8, tag=f"vp{t}", name=f"vp{t}")
              for t in range(NPAIR)]

        # ---------------- helpers ----------------
        def y_proj(blk):
            # yT[e, sq-blk] = A.T @ x.T  (+ w), evac bf16 into yTp plane et
            for et in range(2):
                ps = psum.tile([P, SQB], F32, tag="sc", bufs=4,
                               name=f"py{blk}_{et}")
                for dt in range(2):
                    if blk == 0:
                        rh = head[:, dt * 512: dt * 512 + SQB]
                    else:
                        rh = xTp[:, dt * S + blk * SQB: dt * S + (blk + 1) * SQB]
                    nc.tensor.matmul(
                        ps,
                        lhsT=Apk[:, dt * D + et * P: dt * D + (et + 1) * P],
                        rhs=rh,
                        start=(dt == 0), stop=(dt == 1))
                nc.vector.tensor_scalar_add(
                    out=yTp[:, et * S + blk * SQB: et * S + (blk + 1) * SQB],
                    in0=ps, scalar1=wc[et])

        def v_proj(sk):
            # v'[sk, e'] = x @ (Wv.T R), evac fp8 into vp pair half + ones col
            ps = psum.tile([P, SQB], F32, tag="sc", bufs=4, name=f"pv{sk}")
            for dt in range(2):
                if sk < 4:
                    lh = head[:, dt * 512 + sk * P: dt * 512 + (sk + 1) * P]
                elif sk < 8:
                    lh = head2[:, dt * 512 + (sk - 4) * P:
                               dt * 512 + (sk - 3) * P]
                else:
                    lh = xTp[:, dt * S + sk * P: dt * S + (sk + 1) * P]
                nc.tensor.matmul(
                    ps[:, 0:D], lhsT=lh,
                    rhs=Wpk[:, dt * D:(dt + 1) * D],
                    start=(dt == 0), stop=(dt == 1))
            half = (sk % 2) * VW
            nc.vector.tensor_copy(out=vp[sk // 2][:, half: half + D],
                                  in_=ps[:, 0:D])
            if sk % 2 == 1:
                # both ones-columns of the pair in one strided memset
                nc.gpsimd.memset(_pl_ap(vp[sk // 2], D, 1, VW), 1.0)

        def pv_pair(t, po):
            for sub in range(NSUB):
                lhsT = _pl_ap(pt_pairs[t], sub * P, P, SQB)
                rhs = _pl_ap(vp[t], 0, VW, VW)
                nc.tensor.matmul(
                    po[sub][:, 0:VW], lhsT=lhsT, rhs=rhs, perf_mode=DRM,
                    start=(t == 0), stop=(t == NPAIR - 1))

        def epilogue(po, sub, blk, tail=False):
            # tail mode (last block): num/nT copies run on ScalarE (free
            # after the last exp) so the DVE chain shortens, and out-DMAs
            # avoid gpsimd whose slow SWDGE drain gates the end barrier
            st = blk * NSUB + sub
            rec = epi.tile([P, 1], F32, tag="rec", name=f"rec{st}")
            nc.vector.reciprocal(rec, po[sub][:, D:VW])
            num = epi.tile([P, D], BF16, tag="num", name=f"num{st}")
            if tail:
                nc.scalar.activation(out=num, in_=po[sub][:, 0:D],
                                     func=AF.Copy)
            else:
                nc.vector.tensor_copy(out=num, in_=po[sub][:, 0:D])
            tps = psum.tile([P, 2 * P], BF16, tag=f"o{sub}",
                            name=f"tp{st}")
            for h in range(2):
                nc.tensor.transpose(out=tps[:, h * P:(h + 1) * P],
                                    in_=num[:, h * P:(h + 1) * P],
                                    identity=eye[:, :])
            nT = epi.tile([P, D], BF16, tag="numT", bufs=8, name=f"numT{st}")
            if tail:
                nc.scalar.activation(out=nT, in_=tps, func=AF.Copy)
            else:
                nc.vector.tensor_copy(out=nT, in_=tps)
            # reuse this sub's po bank (freed once rec+num are read) so the
            # epilogue chain never blocks the next block's score rotation
            po2 = psum.tile([P, SQB], F32, tag=f"o{sub}", name=f"po2_{st}")
            for h in range(2):
                nc.tensor.matmul(po2[:, 0:D], lhsT=nT[:, h * P:(h + 1) * P],
                                 rhs=Rtpk[:, h * D:(h + 1) * D],
                                 start=(h == 0), stop=(h == 1))
            osb = epi.tile([P, D], F32, tag="osb", name=f"osb{st}")
            nc.vector.scalar_tensor_tensor(
                out=osb, in0=po2[:, 0:D], scalar=rec, in1=xbt[st],
                op0=mybir.AluOpType.mult, op1=mybir.AluOpType.add)
            if tail:
                oeng = nc.scalar if sub % 2 else nc.sync
            else:
                oeng = nc.gpsimd if sub % 2 else nc.sync
            oeng.dma_start(out=out[st * P:(st + 1) * P, :], in_=osb)

        # ---------------- main loop ----------------
        y_proj(0)
        for blk in range(NBLK):
            po = [psum.tile([P, SQB], F32, tag=f"o{i}", name=f"po{blk}_{i}")
                  for i in range(NSUB)]
            pt_pairs = []
            for sk in range(NSK):
                if blk == 0:
                    v_proj(sk)
                    if sk == 0:
                        nc.gpsimd.dma_start(
                            out=xbig.rearrange("p (t e) -> p t e", t=NSK),
                            in_=xb_src)
                if blk + 1 < NBLK and sk == 10:
                    y_proj(blk + 1)
                if sk % 2 == 0:
                    pt_pairs.append(ptp.tile([P, 2 * SQB], FP8, tag="pt",
                                             name=f"pt{blk}_{sk // 2}"))
                zs = psum.tile([P, SQB], F32, tag="sc", bufs=4,
                               name=f"z{blk}_{sk}")
                for et in range(2):
                    if blk == 0 and sk < 4:
                        lh = head[:, et * 512 + sk * P: et * 512 + (sk + 1) * P]
                    elif blk == 0 and sk < 8:
                        lh = head2[:, et * 512 + (sk - 4) * P:
                                   et * 512 + (sk - 3) * P]
                    else:
                        lh = xTp[:, et * S + sk * P: et * S + (sk + 1) * P]
                    nc.tensor.matmul(
                        zs, lhsT=lh,
                        rhs=yTp[:, et * S + blk * SQB: et * S + (blk + 1) * SQB],
                        start=(et == 0), stop=(et == 1))
                dst = pt_pairs[sk // 2][:, (sk % 2) * SQB:(sk % 2 + 1) * SQB]
                if DVE_EXP and (sk // 2) % 4 == 3:
                    # uint8 exp2 bit-trick on DVE: i = round(z*a + b),
                    # saturating to [0,255]; bit pattern read back as fp8e4.
                    nc.vector.tensor_scalar(
                        out=dst.bitcast(mybir.dt.uint8), in0=zs,
                        scalar1=TRICK_A, scalar2=TRICK_B,
                        op0=mybir.AluOpType.mult, op1=mybir.AluOpType.add)
                else:
                    nc.scalar.activation(
                        out=dst, in_=zs, func=AF.Exp, scale=EXP_SCALE,
                        bias=ebias[:, :])
                if sk % 2 == 1 and sk // 2 >= SKEW:
                    pv_pair(sk // 2 - SKEW, po)
            last = (blk == NBLK - 1)
            if last:
                # close each q-subtile's accumulation group in turn so the
                # epilogue chains pipeline behind the remaining P@V work
                for sub in range(NSUB):
                    for t in range(NPAIR - SKEW, NPAIR):
                        lhsT = _pl_ap(pt_pairs[t], sub * P, P, SQB)
                        rhs = _pl_ap(vp[t], 0, VW, VW)
                        nc.tensor.matmul(
                            po[sub][:, 0:VW], lhsT=lhsT, rhs=rhs,
                            perf_mode=DRM, start=False,
                            stop=(t == NPAIR - 1))
            else:
                for t in range(NPAIR - SKEW, NPAIR):
                    pv_pair(t, po)
            for sub in range(NSUB):
                epilogue(po, sub, blk, tail=last)

    nc.finalize()
    _NC_CACHE = nc
    return nc


def _hadamard(n):
    H = np.array([[1.0]], dtype=np.float64)
    while H.shape[0] < n:
        H = np.block([[H, H], [H, -H]])
    return H


def _pack(M):
    """[256, 256] -> [128, 512]: out[i, dt*256+e] = M[dt*128+i, e]."""
    return np.concatenate([M[0:P, :], M[P:2 * P, :]], axis=1)


def _run(inputs, **spmd_kwargs):
    nc = _build()
    bf = ml_dtypes.bfloat16
    x = np.asarray(inputs["x"], dtype=np.float32)
    Wq = np.asarray(inputs["Wq"], dtype=np.float32)
    Wk = np.asarray(inputs["Wk"], dtype=np.float32)
    Wv = np.asarray(inputs["Wv"], dtype=np.float32)
    bq = np.asarray(inputs["bq"], dtype=np.float32)
    bv = np.asarray(inputs["bv"], dtype=np.float32)

    scale16 = 16.0 / np.sqrt(D)
    A = (scale16 * (Wq.T @ Wk)).astype(np.float32)
    wcol = (scale16 * (bq @ Wk)).astype(np.float32)
    R = (_hadamard(D) / 16.0).astype(np.float32)
    WvR = (Wv.T @ R).astype(np.float32)
    Rt = np.ascontiguousarray(R.T)

    eye = np.eye(P, dtype=np.float32)
    Apk_h = _pack(A).astype(bf)
    shared = {
        "eye": np.ascontiguousarray(eye.astype(bf)),
        "Apk": np.ascontiguousarray(Apk_h),
        "Wpk": np.ascontiguousarray(_pack(WvR).astype(bf)),
        "Rtpk": np.ascontiguousarray(_pack(Rt).astype(bf)),
        "wcol": wcol,
    }
    in_maps = []
    for i in range(B):
        xT = np.ascontiguousarray(x[i].T.astype(bf))  # [256, 4096]
        xTpk = np.concatenate([xT[0:P], xT[P:2 * P]], axis=1)
        m = {"xb": np.ascontiguousarray(x[i] + bv),
             "xT": np.ascontiguousarray(xTpk),
             "head": np.ascontiguousarray(np.concatenate(
                 [xTpk[:, 0:512], xTpk[:, S:S + 512], Apk_h], axis=1)),
             "head2": np.ascontiguousarray(np.concatenate(
                 [xTpk[:, 512:1024], xTpk[:, S + 512:S + 1024]], axis=1)),
             **shared}
        in_maps.append(m)
    res = run_bass_kernel_spmd(nc, in_maps, core_ids=list(range(B)),
                               **spmd_kwargs)
    full = np.stack([r["out"] for r in res.results], axis=0)
    return full, res


def kernel(**inputs):
    return _run(inputs)[0]


# revision 22
# speedup vs baseline: 1.0194x; 1.0194x over previous
"""Single-head cross-attention block on 8 NeuronCores (Trainium2, Bass/Tile).

Problem:  out = x + softmax((x@Wq.T+bq) @ (x@Wk.T+bk).T / sqrt(D)) @ (x@Wv.T+bv)
          x: [8, 4096, 256] f32.  Data-parallel: one batch element per core.

Design (see kernel_base.py for the 219-221us baseline rationale; this
revision keeps its numerics bit-identical and attacks PE idle):
  1. Scores algebra: z = (x@A + w) @ x.T, A = 16*scale*Wq.T@Wk host-folded.
  2. P@V in fp8e4 DoubleRow; P = exp(z/16 - C*ln2) straight to fp8; row-sum
     rides as a ones-column in v.  v fp8 in a Hadamard-rotated basis.
  3. exp split 3:1 ScalarE (exact) : VectorE (uint8 Schraudolph exp2).
  4. Trace-driven changes vs baseline (each kills a measured PE stall):
     - head DMA doubled to xT cols 0:1024 both planes (+Apk): blk0's PE
       work no longer outruns the xT chunk DMAs at sk=4..7 (was a 3.7us
       LDWEIGHTS stall at t~7-11us); remaining chunks reordered 1024:2560
       first, 0:1024 refill last.
     - y_proj(blk+1) issued at sk==10 (was 16): its PSUM slots sit in the
       score rotation ring, and at sk==16 the DVE evac queue is long, so
       pair MMs stalled ~1.6us/block waiting the y evac.  At sk==10 the
       DVE queue is empty (pair 3's exp just drained).
     - output DMA triggers moved off the ScalarE queue (sync+gpsimd now):
       a 619ns trigger between exp ACTIVATEs delayed evacs -> rotation
       stalls.
     - exp act-table warmup at t~0 (ebias memset on the vector queue, which
       is free early; gpsimd has ~4us of framework preamble).
     - vp ones-columns: one 2-plane strided memset per pair (was 2).
"""

import numpy as np
import ml_dtypes
from contextlib import ExitStack

import concourse.bass as bass
import concourse.mybir as mybir
import concourse.tile as tile
from concourse import bacc
from concourse.bass_utils import run_bass_kernel_spmd

B, S, D = 8, 4096, 256
P = 128
SQB = 512               # sq block width
NBLK = S // SQB         # 8
NSK = S // P            # 32 sk tiles
NPAIR = NSK // 2        # 16 fp8 DoubleRow pairs
NSUB = SQB // P         # 4
VW = D + 1              # v columns + ones column
SKEW = 3                # PV pair pipeline skew (in pairs)
HEADC = 1024            # head-DMA xT columns per plane
C_SHIFT = 7.4           # P = 2^(z' - C)
EXP_SCALE = 1.0 / 16.0
EXP_BIAS = float(-C_SHIFT * np.log(2.0))
L2E = 1.4426950408889634
TRICK_A = 8.0 * L2E / 16.0                  # uint8 exp2 bit-trick slope
TRICK_B = 56.0 - 8.0 * C_SHIFT - 0.344      # offset, mean-error-centered

F32 = mybir.dt.float32
BF16 = mybir.dt.bfloat16
FP8 = mybir.dt.float8e4
DRM = mybir.MatmulPerfMode.DoubleRow
AF = mybir.ActivationFunctionType

DVE_EXP = True
_NC_CACHE = None


def _col_ap(vec_ap):
    return bass.AP(tensor=vec_ap.tensor, offset=vec_ap.offset,
                   ap=[vec_ap.ap[0], [0, 1]])


def _pl_ap(t, off, n, plane_stride):
    """[128, 2, n] AP over tile t: planes at element offsets off, off+plane_stride."""
    base = t[:, 0:1]
    return bass.AP(tensor=base.tensor, offset=base.offset + off,
                   ap=[base.ap[0], [plane_stride, 2], [1, n]])


def _build():
    global _NC_CACHE
    if _NC_CACHE is not None:
        return _NC_CACHE

    nc = bacc.Bacc("TRN2")
    xb = nc.dram_tensor("xb", [S, D], F32, kind="ExternalInput")      # x + bv
    xTh = nc.dram_tensor("xT", [P, 2 * S], BF16, kind="ExternalInput")  # packed x.T
    Ah = nc.dram_tensor("Apk", [P, 2 * D], BF16, kind="ExternalInput")
    Wh = nc.dram_tensor("Wpk", [P, 2 * D], BF16, kind="ExternalInput")  # Wv.T @ R
    Rh = nc.dram_tensor("Rtpk", [P, 2 * D], BF16, kind="ExternalInput")  # R.T
    wh = nc.dram_tensor("wcol", [D], F32, kind="ExternalInput")
    eyeh = nc.dram_tensor("eye", [P, P], BF16, kind="ExternalInput")

    out = nc.dram_tensor("out", [S, D], F32, kind="ExternalOutput")

    with tile.TileContext(nc) as tc, ExitStack() as ctx:
        persist = ctx.enter_context(tc.tile_pool(name="persist", bufs=1))
        psum = ctx.enter_context(tc.tile_pool(name="psum", bufs=1, space="PSUM"))
        ptp = ctx.enter_context(tc.tile_pool(name="ptp", bufs=8))
        epi = ctx.enter_context(tc.tile_pool(name="epi", bufs=8))

        # --- warmup: pull the exp act-table load (~1.3us) into the DMA
        #     shadow at t=0 ---
        ebias = persist.tile([P, 1], F32, tag="ebias", name="ebias")
        nc.vector.memset(ebias, EXP_BIAS)
        wro = persist.tile([P, 1], F32, tag="wro", name="wro")
        nc.scalar.activation(out=wro, in_=ebias, func=AF.Exp,
                             scale=EXP_SCALE, bias=ebias[:, :])

        # --- startup-critical DMAs.  Engine preambles delay the first
        #     trigger to ~6.5us, desc-gen costs ~0.65us per trigger, and
        #     each HWDGE ring drains its transfers FIFO.  So: the smallest
        #     needed-first transfer (Apk, 0.25MB) leads the sync ring,
        #     followed by xT chunks in exact PE-consumption order, with the
        #     per-plane partners split across the sync/scalar rings ---
        xTp = persist.tile([P, 2 * S], BF16, tag="xTp", name="xTp")

        def xpart(eng, pl, lo, hi):
            eng.dma_start(out=xTp[:, pl * S + lo: pl * S + hi],
                          in_=xTh[:, pl * S + lo: pl * S + hi])

        Apkt = persist.tile([P, 2 * D], BF16, tag="Apkt", name="Apkt")
        nc.sync.dma_start(out=Apkt, in_=Ah[:, :])
        Apk = Apkt[:, :]
        xpart(nc.sync, 0, 0, 512)
        xpart(nc.sync, 1, 0, 512)
        Wpk = persist.tile([P, 2 * D], BF16, tag="Wpk", name="Wpk")
        nc.scalar.dma_start(out=Wpk, in_=Wh[:, :])
        wc = []
        for et in range(2):
            t = persist.tile([P, 1], F32, tag=f"wc{et}", name=f"wc{et}")
            nc.scalar.dma_start(out=t, in_=_col_ap(wh[et * P:(et + 1) * P]))
            wc.append(t)
        xpart(nc.sync, 0, 512, 1024)
        xpart(nc.sync, 1, 512, 1024)
        xpart(nc.scalar, 1, 1024, 2560)
        xpart(nc.sync, 0, 1024, 2560)
        xpart(nc.sync, 0, 2560, S)
        xpart(nc.scalar, 1, 2560, S)
        Rtpk = persist.tile([P, 2 * D], BF16, tag="Rtpk", name="Rtpk")
        nc.scalar.dma_start(out=Rtpk, in_=Rh[:, :])
        eye = persist.tile([P, P], BF16, tag="eye", name="eye")
        nc.gpsimd.dma_start(out=eye, in_=eyeh[:, :])

        # --- residual (x + bv): one big tile, one SWDGE transfer (issued at
        #     blk0 sk=0 so it doesn't clog startup descriptor generation) ---
        xbig = persist.tile([P, NSK * D], F32, tag="xbig", name="xbig")
        xb_col = xb[0:P, 0:1]
        xb_src = bass.AP(tensor=xb_col.tensor, offset=xb_col.offset,
                         ap=[xb_col.ap[0], [P * D, NSK], [1, D]])
        xbt = [xbig[:, st * D:(st + 1) * D] for st in range(NSK)]

        yTp = persist.tile([P, 2 * S], BF16, tag="yTp", name="yTp")
        vp = [persist.tile([P, 2 * VW], FP8, tag=f"vp{t}", name=f"vp{t}")
              for t in range(NPAIR)]

        # ---------------- helpers ----------------
        def y_proj(blk):
            # yT[e, sq-blk] = A.T @ x.T  (+ w), evac bf16 into yTp plane et
            for et in range(2):
                ps = psum.tile([P, SQB], F32, tag="sc", bufs=4,
                               name=f"py{blk}_{et}")
                for dt in range(2):
                    rh = xTp[:, dt * S + blk * SQB: dt * S + (blk + 1) * SQB]
                    nc.tensor.matmul(
                        ps,
                        lhsT=Apk[:, dt * D + et * P: dt * D + (et + 1) * P],
                        rhs=rh,
                        start=(dt == 0), stop=(dt == 1))
                nc.vector.tensor_scalar_add(
                    out=yTp[:, et * S + blk * SQB: et * S + (blk + 1) * SQB],
                    in0=ps, scalar1=wc[et])

        def v_proj(sk):
            # v'[sk, e'] = x @ (Wv.T R), evac fp8 into vp pair half + ones col
            ps = psum.tile([P, SQB], F32, tag="sc", bufs=4, name=f"pv{sk}")
            for dt in range(2):
                lh = xTp[:, dt * S + sk * P: dt * S + (sk + 1) * P]
                nc.tensor.matmul(
                    ps[:, 0:D], lhsT=lh,
                    rhs=Wpk[:, dt * D:(dt + 1) * D],
                    start=(dt == 0), stop=(dt == 1))
            half = (sk % 2) * VW
            nc.vector.tensor_copy(out=vp[sk // 2][:, half: half + D],
                                  in_=ps[:, 0:D])
            if sk % 2 == 1:
                # both ones-columns of the pair in one strided memset
                nc.gpsimd.memset(_pl_ap(vp[sk // 2], D, 1, VW), 1.0)

        def pv_pair(t, po):
            for sub in range(NSUB):
                lhsT = _pl_ap(pt_pairs[t], sub * P, P, SQB)
                rhs = _pl_ap(vp[t], 0, VW, VW)
                nc.tensor.matmul(
                    po[sub][:, 0:VW], lhsT=lhsT, rhs=rhs, perf_mode=DRM,
                    start=(t == 0), stop=(t == NPAIR - 1))

        def epilogue(po, sub, blk, tail=False):
            # tail mode (last block): num/nT copies run on ScalarE (free
            # after the last exp) so the DVE chain shortens, and out-DMAs
            # avoid gpsimd whose slow SWDGE drain gates the end barrier
            st = blk * NSUB + sub
            rec = epi.tile([P, 1], F32, tag="rec", name=f"rec{st}")
            nc.vector.reciprocal(rec, po[sub][:, D:VW])
            num = epi.tile([P, D], BF16, tag="num", name=f"num{st}")
            if tail:
                nc.scalar.activation(out=num, in_=po[sub][:, 0:D],
                                     func=AF.Copy)
            else:
                nc.vector.tensor_copy(out=num, in_=po[sub][:, 0:D])
            tps = psum.tile([P, 2 * P], BF16, tag=f"o{sub}",
                            name=f"tp{st}")
            for h in range(2):
                nc.tensor.transpose(out=tps[:, h * P:(h + 1) * P],
                                    in_=num[:, h * P:(h + 1) * P],
                                    identity=eye[:, :])
            nT = epi.tile([P, D], BF16, tag="numT", bufs=8, name=f"numT{st}")
            if tail:
                nc.scalar.activation(out=nT, in_=tps, func=AF.Copy)
            else:
                nc.vector.tensor_copy(out=nT, in_=tps)
            # reuse this sub's po bank (freed once rec+num are read) so the
            # epilogue chain never blocks the next block's score rotation
            po2 = psum.tile([P, SQB], F32, tag=f"o{sub}", name=f"po2_{st}")
            for h in range(2):
                nc.tensor.matmul(po2[:, 0:D], lhsT=nT[:, h * P:(h + 1) * P],
                                 rhs=Rtpk[:, h * D:(h + 1) * D],
                                 start=(h == 0), stop=(h == 1))
            osb = epi.tile([P, D], F32, tag="osb", name=f"osb{st}")
            nc.vector.scalar_tensor_tensor(
                out=osb, in0=po2[:, 0:D], scalar=rec, in1=xbt[st],
                op0=mybir.AluOpType.mult, op1=mybir.AluOpType.add)
            if tail:
                oeng = nc.scalar if sub % 2 else nc.sync
            else:
                oeng = nc.gpsimd if sub % 2 else nc.sync
            oeng.dma_start(out=out[st * P:(st + 1) * P, :], in_=osb)

        # ---------------- main loop ----------------
        y_proj(0)
        for blk in range(NBLK):
            po = [psum.tile([P, SQB], F32, tag=f"o{i}", name=f"po{blk}_{i}")
                  for i in range(NSUB)]
            pt_pairs = []
            for sk in range(NSK):
                if blk == 0:
                    v_proj(sk)
                    if sk == 24:
                        # residual DMA deferred past the startup window so it
                        # doesn't steal HBM bandwidth from the xT chunks;
                        # blk0's epilogue needs xbt[0..3] at ~31us, tile 0
                        # arrives ~25us
                        nc.gpsimd.dma_start(
                            out=xbig.rearrange("p (t e) -> p t e", t=NSK),
                            in_=xb_src)
                if blk + 1 < NBLK and sk == 10:
                    y_proj(blk + 1)
                if sk % 2 == 0:
                    pt_pairs.append(ptp.tile([P, 2 * SQB], FP8, tag="pt",
                                             name=f"pt{blk}_{sk // 2}"))
                zs = psum.tile([P, SQB], F32, tag="sc", bufs=4,
                               name=f"z{blk}_{sk}")
                for et in range(2):
                    lh = xTp[:, et * S + sk * P: et * S + (sk + 1) * P]
                    nc.tensor.matmul(
                        zs, lhsT=lh,
                        rhs=yTp[:, et * S + blk * SQB: et * S + (blk + 1) * SQB],
                        start=(et == 0), stop=(et == 1))
                dst = pt_pairs[sk // 2][:, (sk % 2) * SQB:(sk % 2 + 1) * SQB]
                if DVE_EXP and (sk // 2) % 4 == 3:
                    # uint8 exp2 bit-trick on DVE: i = round(z*a + b),
                    # saturating to [0,255]; bit pattern read back as fp8e4.
                    nc.vector.tensor_scalar(
                        out=dst.bitcast(mybir.dt.uint8), in0=zs,
                        scalar1=TRICK_A, scalar2=TRICK_B,
                        op0=mybir.AluOpType.mult, op1=mybir.AluOpType.add)
                else:
                    nc.scalar.activation(
                        out=dst, in_=zs, func=AF.Exp, scale=EXP_SCALE,
                        bias=ebias[:, :])
                if sk % 2 == 1 and sk // 2 >= SKEW:
                    pv_pair(sk // 2 - SKEW, po)
            last = (blk == NBLK - 1)
            if last:
                # close each q-subtile's accumulation group in turn so the
                # epilogue chains pipeline behind the remaining P@V work
                for sub in range(NSUB):
                    for t in range(NPAIR - SKEW, NPAIR):
                        lhsT = _pl_ap(pt_pairs[t], sub * P, P, SQB)
                        rhs = _pl_ap(vp[t], 0, VW, VW)
                        nc.tensor.matmul(
                            po[sub][:, 0:VW], lhsT=lhsT, rhs=rhs,
                            perf_mode=DRM, start=False,
                            stop=(t == NPAIR - 1))
            else:
                for t in range(NPAIR - SKEW, NPAIR):
                    pv_pair(t, po)
            for sub in range(NSUB):
                epilogue(po, sub, blk, tail=last)

    nc.finalize()
    _NC_CACHE = nc
    return nc


def _hadamard(n):
    H = np.array([[1.0]], dtype=np.float64)
    while H.shape[0] < n:
        H = np.block([[H, H], [H, -H]])
    return H


def _pack(M):
    """[256, 256] -> [128, 512]: out[i, dt*256+e] = M[dt*128+i, e]."""
    return np.concatenate([M[0:P, :], M[P:2 * P, :]], axis=1)


def _run(inputs, **spmd_kwargs):
    nc = _build()
    bf = ml_dtypes.bfloat16
    x = np.asarray(inputs["x"], dtype=np.float32)
    Wq = np.asarray(inputs["Wq"], dtype=np.float32)
    Wk = np.asarray(inputs["Wk"], dtype=np.float32)
    Wv = np.asarray(inputs["Wv"], dtype=np.float32)
    bq = np.asarray(inputs["bq"], dtype=np.float32)
    bv = np.asarray(inputs["bv"], dtype=np.float32)

    scale16 = 16.0 / np.sqrt(D)
    A = (scale16 * (Wq.T @ Wk)).astype(np.float32)
    wcol = (scale16 * (bq @ Wk)).astype(np.float32)
    R = (_hadamard(D) / 16.0).astype(np.float32)
    WvR = (Wv.T @ R).astype(np.float32)
    Rt = np.ascontiguousarray(R.T)

    eye = np.eye(P, dtype=np.float32)
    Apk_h = _pack(A).astype(bf)
    shared = {
        "eye": np.ascontiguousarray(eye.astype(bf)),
        "Apk": np.ascontiguousarray(Apk_h),
        "Wpk": np.ascontiguousarray(_pack(WvR).astype(bf)),
        "Rtpk": np.ascontiguousarray(_pack(Rt).astype(bf)),
        "wcol": wcol,
    }
    in_maps = []
    for i in range(B):
        xT = np.ascontiguousarray(x[i].T.astype(bf))  # [256, 4096]
        xTpk = np.concatenate([xT[0:P], xT[P:2 * P]], axis=1)
        m = {"xb": np.ascontiguousarray(x[i] + bv),
             "xT": np.ascontiguousarray(xTpk),
             **shared}
        in_maps.append(m)
    res = run_bass_kernel_spmd(nc, in_maps, core_ids=list(range(B)),
                               **spmd_kwargs)
    full = np.stack([r["out"] for r in res.results], axis=0)
    return full, res


def kernel(**inputs):
    return _run(inputs)[0]
